# revision 22
# baseline (speedup 1.0000x reference)
"""Causal self-attention (B=1, L=4096, C=1024, H=16, D=64) on 8 TRN2 NeuronCores.

Sharding: head tensor-parallelism - each core owns 2 of the 16 heads and
computes a full [C, L] partial of out.T = Wo_local.T @ attn_local.T; the
host sums the 8 partials.

v2: the v1 trace showed the Scalar engine (softmax exp) 97% busy in the
attention steady state while the PE idled between S and PV waiting on
exp - and the PE's DVFS ramp reset at every gap, capping matmuls at
~1.35 GHz instead of 2.4. Fixes:

  * EXP2BF16_ANT custom DVE op: computes the BF16 *bit pattern* of
    K*exp(score/8) in one 8-stage pass (magic-constant floor, quadratic
    2^f mantissa correction, int16 convert-on-write into a bf16-aliased
    tile; the constant K cancels in the per-head softmax
    normalization). Head 0's exps stay on the Scalar engine
    (exp activation), head 1's run on the DVE -> each engine carries
    half the softmax load and both fit under the PE's per-pair time.
    qt2 is pre-scaled by 16*log2e so the S-matmul psum is already in
    128*log2-units for the DVE op; the scalar path compensates with
    scale=ln2/128.
  * PV pending depth 2 + per-j tails (normalize + Wo + out-DMA) emitted
    between pair 0's exps and the first PV of the next j-block, so the
    in-order PE queue always has S-work buffered ahead of the
    recip/normalize latency chain.
  * ot psum->bf16 casts moved DVE -> Scalar (Copy shares the act table
    with Exp: no table reloads), balancing the three engines.
  * merged normalize: one reciprocal over both heads' denominators, one
    [2,128]-blkones broadcast matmul for both heads.
"""
import math
import sys
from contextlib import ExitStack

import numpy as np

sys.path.insert(0, "/opt/trn_rl_repo")

import ml_dtypes  # noqa: E402

import concourse.bass as bass  # noqa: E402,F401
import concourse.mybir as mybir  # noqa: E402
import concourse.tile as tile  # noqa: E402
from concourse import bacc  # noqa: E402

FP32 = mybir.dt.float32
FP16 = mybir.dt.float16
BF16 = mybir.dt.bfloat16
I16 = mybir.dt.int16

L, C, H, D = 4096, 1024, 16, 64
N_CORES = 8
EXP_BIAS = -2.0

# --- custom DVE op: bf16 bits of 2^(w/128) * 2^(B7/128 - 127.5) ------------
QSCALE = 16.0 * 1.4426950408889634          # qt2 prescale: st = score*128*log2e/8
SC_SCALE = math.log(2.0) / 128.0            # scalar-engine exp scale on st
EXP_C0 = float(np.float32(1.5 * 2 ** 30))   # magic floor constant
EXP_C1 = float(np.float32(0.002687499626))  # b2/128 (quadratic 2^f corr)
EXP_C2 = float(np.float32(-0.004968570545))  # b1
# 127*128 + 64 + b0 - 2*log2e*128: the -369.33 folds the exp(-2) bias in
# (keeps 1/denominator comfortably inside fp16-normal range downstream).
EXP_B7 = float(np.float32(16308.992188 - 369.32986))


def _exp_ref(in0, in1, s0, s1, imm2):
    w = in0.astype(np.float32)
    r = (w + np.float32(s0)).astype(np.float32)
    k7 = (r - np.float32(s0)).astype(np.float32)
    f7 = (w - k7).astype(np.float32)
    u3 = (((f7 * np.float32(s1)).astype(np.float32)
           + np.float32(imm2)).astype(np.float32) * f7).astype(np.float32)
    return ((w + u3).astype(np.float32) + in1.astype(np.float32)).astype(np.float32)


def _register_exp_op():
    from concourse import dve_ops as _dvo
    from concourse.dve_spec import Spec, Src0, Src1, C0, C1, C2, Latch, lower
    from concourse.dve_uop import DveOpSpec

    name = "EXP2BF16_ANT"
    if name in _dvo._SUB_OPCODE_FOR_NAME:
        return next(op for op in _dvo.OPS if op.name == name)
    r = Src0 + C0
    k7 = r - C0
    f7 = Src0 - k7
    u3 = (f7 * C1 + C2) * f7
    body = (Src0 + u3) + Latch(Src1)
    spec = Spec(body=body, reference=_exp_ref)
    shas = {}
    for ver in ("v3", "v4"):
        shas[ver] = DveOpSpec(name=name, opcode=0,
                              uops=lower(spec, ver=ver), rd1_en=True).sha(ver)
    op = _dvo.DveOp(name, spec, subdim=False, uops_sha=shas)
    _dvo.OPS.append(op)
    _dvo._SUB_OPCODE_FOR_NAME[name] = (
        _dvo._CUSTOM_DVE_ROW_BASE + len(_dvo.OPS) - 1)
    _dvo.CUSTOM_DVE_SPECS[name] = spec
    return op


EXP_OP = _register_exp_op()


def _build_nc():
    DH2, QB, KB = 128, 512, 128
    NQ = L // QB          # 8 q-blocks
    NCC = C // 128        # 8 contraction chunks
    SUB = QB // KB        # 4 k-blocks per q-block width
    NKB = L // KB         # 32 k-blocks
    Exp = mybir.ActivationFunctionType.Exp
    Copy = mybir.ActivationFunctionType.Copy

    nc = bacc.Bacc("TRN2", target_bir_lowering=False, debug=False,
                   num_devices=N_CORES)
    xT = nc.declare_dram_parameter("xT", [C, L], BF16, isOutput=False)
    wq = nc.declare_dram_parameter("wq", [128, C], BF16, isOutput=False)
    wk = nc.declare_dram_parameter("wk", [128, C], BF16, isOutput=False)
    wv = nc.declare_dram_parameter("wv", [128, C], BF16, isOutput=False)
    wo = nc.declare_dram_parameter("wo", [DH2, C], BF16, isOutput=False)
    outT = nc.declare_dram_parameter("outT", [C, L], BF16, isOutput=True)

    xT_v = xT.rearrange("(n p) l -> p n l", n=NCC)
    outT_v = outT.rearrange("(n p) l -> p n l", n=NCC)

    with tile.TileContext(nc) as tc, ExitStack() as ctx:
        big = ctx.enter_context(tc.tile_pool(name="big", bufs=1))
        work = ctx.enter_context(tc.tile_pool(name="work", bufs=4))
        psA = ctx.enter_context(tc.tile_pool(name="psA", bufs=2, space="PSUM"))
        psS = ctx.enter_context(tc.tile_pool(name="psS", bufs=2, space="PSUM"))
        psO = ctx.enter_context(tc.tile_pool(name="psO", bufs=1, space="PSUM"))

        ident = big.tile([128, 128], BF16, tag="ident")
        nc.gpsimd.memset(ident[:], 0.0)
        nc.gpsimd.affine_select(out=ident[:], in_=ident[:],
                                compare_op=mybir.AluOpType.not_equal,
                                fill=1.0, base=0,
                                pattern=[[-1, 128]], channel_multiplier=1)

        ebias = big.tile([128, 1], FP32, tag="ebias")
        nc.gpsimd.memset(ebias[:], EXP_BIAS)
        b7 = big.tile([128, 1], FP32, tag="b7")
        nc.gpsimd.memset(b7[:], EXP_B7)

        # weights FIRST (the proj of block 0 needs them; a weights-last order
        # was measured to stall the first matmul until t=35us), then x block
        # 0 split in halves, then the rest of x.
        wq_sb = big.tile([128, NCC, DH2], BF16, tag="wq")
        wk_sb = big.tile([128, NCC, DH2], BF16, tag="wk")
        wv_sb = big.tile([128, NCC, DH2], BF16, tag="wv")
        # alternate DMA triggers between the Sync and Scalar HWDGE queues so
        # the weight and x streams transfer in parallel
        for eng, (w_sb, w_dram) in zip(
                (nc.sync, nc.scalar, nc.sync),
                ((wq_sb, wq), (wk_sb, wk), (wv_sb, wv))):
            eng.dma_start(w_sb[:], w_dram.rearrange("p (n d) -> p n d", n=NCC))
        wo_sb = big.tile([128, C], BF16, tag="wo")
        nc.scalar.dma_start(wo_sb[:], wo[:])
        xt_sb = big.tile([128, NCC, L], BF16, tag="xt")
        for b in (0, 1):
            for h in range(2):
                cc = slice(h * (NCC // 2), (h + 1) * (NCC // 2))
                eng = nc.sync if h == 0 else nc.scalar
                eng.dma_start(xt_sb[:, cc, b * QB:(b + 1) * QB],
                              xT_v[:, cc, b * QB:(b + 1) * QB])
        for b in range(2, NQ):
            eng = nc.sync if b % 2 == 0 else nc.scalar
            eng.dma_start(xt_sb[:, :, b * QB:(b + 1) * QB],
                          xT_v[:, :, b * QB:(b + 1) * QB])

        qt2 = big.tile([128, L], FP16, tag="qt2")
        kt2 = big.tile([128, L], FP16, tag="kt2")
        vt2 = big.tile([128, L], BF16, tag="vt2")
        vaug = big.tile([128, NKB, 130], BF16, tag="vaug")
        nc.gpsimd.memset(vaug[:, :, 64:65], 1.0)
        nc.gpsimd.memset(vaug[:, :, 129:130], 1.0)

        def emit_vtrans(b):
            for i in range(b * SUB, (b + 1) * SUB):
                trp = psA.tile([128, KB], BF16, tag="ps")
                nc.tensor.transpose(trp[:], vt2[:, i * KB:(i + 1) * KB],
                                    ident[:])
                nc.vector.tensor_copy(vaug[:, i, 0:64], trp[:, 0:64])
                nc.vector.tensor_copy(vaug[:, i, 65:129], trp[:, 64:128])

        for b in range(NQ):
            cols = slice(b * QB, (b + 1) * QB)
            for dst, w_sb, qs in ((qt2, wq_sb, True), (kt2, wk_sb, False),
                                  (vt2, wv_sb, False)):
                pp = psA.tile([128, QB], FP32, tag="ps")
                for c in range(NCC):
                    nc.tensor.matmul(pp[:], w_sb[:, c, :],
                                     xt_sb[:, c, cols],
                                     start=(c == 0), stop=(c == NCC - 1))
                nc.scalar.activation(dst[:, cols], pp[:], Copy,
                                     scale=QSCALE if qs else 1.0)
            if b > 0:
                emit_vtrans(b - 1)
        emit_vtrans(NQ - 1)

        pending_tail = [None]  # deferred (j, att2, o_ps) normalize+Wo+DMA

        def emit_tail(last=False):
            if pending_tail[0] is None:
                return
            j, att2, o_ps = pending_tail[0]
            pending_tail[0] = None
            dn2 = work.tile([1, 2, QB], FP32, tag="dn")
            for h in range(2):
                nc.vector.tensor_copy(dn2[0:1, h, :], o_ps[h][64:65, :])
            recip = work.tile([1, 2, QB], FP32, tag="recip")
            nc.vector.reciprocal_approx_fast(recip[:], dn2[:])
            for h in range(2):
                r0, r1 = h * D, (h + 1) * D
                # gpsimd column-broadcast of 1/denominator across the 64
                # head dims (replaces a PE ones-outer-product + DVE copy)
                bc_sb = work.tile([64, QB], FP32, tag=f"bc{h}")
                nc.gpsimd.partition_broadcast(bc_sb[:], recip[0:1, h, :])
                nc.vector.tensor_mul(att2[r0:r1, :], o_ps[h][0:64, :],
                                     bc_sb[:])
            ot = work.tile([128, NCC, QB], BF16, tag="ot", bufs=2)
            for cc in range(NCC):
                op = psA.tile([128, QB], FP32, tag="ps")
                nc.tensor.matmul(op[:], wo_sb[:, cc * 128:(cc + 1) * 128],
                                 att2[:], start=True, stop=True)
                if last and cc % 2 == 1:
                    nc.vector.tensor_copy(ot[:, cc, :], op[:])
                else:
                    nc.scalar.activation(ot[:, cc, :], op[:], Copy)
            nc.sync.dma_start(outT_v[:, :, j * QB:(j + 1) * QB], ot[:])

        def emit_pair_S(j, p):
            """S matmuls + exps + causal masking for pair p of block j;
            returns the pts tiles for the later PV emission."""
            c0s = [max(0, (2 * p + s - j * SUB)) * KB for s in range(2)]
            pts = []
            for h in range(2):
                r0, r1 = h * D, (h + 1) * D
                st = psS.tile([128, 2, QB], FP32, tag="st", name="st")
                for s in range(2):
                    i = 2 * p + s
                    nc.tensor.matmul(
                        st[:, s, c0s[s]:QB],
                        kt2[r0:r1, i * KB:(i + 1) * KB],
                        qt2[r0:r1, j * QB + c0s[s]:(j + 1) * QB],
                        start=True, stop=True)
                pt = work.tile([128, 2, QB], BF16, tag=f"pt{h}", bufs=5,
                               name=f"pt{h}")
                # one exp call over both slabs when neither is clipped
                slabs = ([(slice(0, 2), 0)] if c0s[0] == c0s[1] == 0
                         else [(slice(s, s + 1), c0s[s]) for s in range(2)])
                for ss, c0 in slabs:
                    if h == 0:
                        nc.scalar.activation(pt[:, ss, c0:QB],
                                             st[:, ss, c0:QB], Exp,
                                             bias=ebias[:], scale=SC_SCALE)
                    else:
                        nc.vector._custom_dve(
                            EXP_OP,
                            out=pt[:, ss, c0:QB].bitcast(I16),
                            in0=st[:, ss, c0:QB], in1=b7[:],
                            s0=EXP_C0, s1=EXP_C1, imm2=EXP_C2)
                for s in range(2):
                    if 2 * p + s >= j * SUB:
                        c0 = c0s[s]
                        if c0 > 0:
                            nc.gpsimd.memset(pt[:, s, 0:c0], 0.0)
                        nc.gpsimd.affine_select(
                            out=pt[:, s, c0:c0 + KB],
                            in_=pt[:, s, c0:c0 + KB],
                            compare_op=mybir.AluOpType.is_ge, fill=0.0,
                            base=0, pattern=[[1, KB]],
                            channel_multiplier=-1)
                pts.append(pt)
            return pts

        carry = [None]  # next j's pair-0 pts, pre-rolled at j-1's flush
        for j in range(NQ):
            att2 = work.tile([128, QB], BF16, tag="att2", bufs=2)
            o_ps = [psO.tile([65, QB], FP32, tag=f"o{h}", name=f"o_ps{h}")
                    for h in range(2)]
            nk = (j + 1) * SUB
            npair = nk // 2
            pending = []  # [(pts, pair_idx), ...] awaiting PV emission

            def emit_pv(pts, p, o_ps=o_ps, nk=nk):
                for h in range(2):
                    for s in range(2):
                        i = 2 * p + s
                        nc.tensor.matmul(
                            o_ps[h][:, :],
                            vaug[:, i, 65 * h:65 * h + 65],
                            pts[h][:, s, :],
                            start=(i == 0), stop=(i == nk - 1))

            for p in range(npair):
                if p == 0 and carry[0] is not None:
                    pts = carry[0]
                    carry[0] = None
                else:
                    pts = emit_pair_S(j, p)
                pending.append((pts, p))
                if p == min(1, npair - 1):
                    emit_tail()  # previous j's normalize+Wo before first PV
                if len(pending) > 2:
                    emit_pv(*pending.pop(0))
            # pre-roll the next block's first S+exp pair so the PE and the
            # exp engines stay fed across the j boundary
            if j + 1 < NQ:
                carry[0] = emit_pair_S(j + 1, 0)
            while pending:
                emit_pv(*pending.pop(0))
            pending_tail[0] = (j, att2, o_ps)
        emit_tail(last=True)
    nc.compile()
    return nc


_NC_CACHE = None


def _get_nc():
    global _NC_CACHE
    if _NC_CACHE is None:
        _NC_CACHE = _build_nc()
    return _NC_CACHE


def _chunk_major(w):
    """[1024, 128] -> [128, 8*128]: element [p, n*128+d] = w[n*128+p, d]."""
    return np.ascontiguousarray(
        w.reshape(8, 128, 128).transpose(1, 0, 2).reshape(128, 1024))


def make_in_maps(x, Wq, Wk, Wv, Wo):
    bf16 = ml_dtypes.bfloat16
    x = np.asarray(x, np.float32).reshape(L, C)
    xT = np.ascontiguousarray(x.T).astype(bf16)
    Wq, Wk, Wv, Wo = (np.asarray(w, np.float32) for w in (Wq, Wk, Wv, Wo))
    in_maps = []
    for c in range(N_CORES):
        cols = slice(128 * c, 128 * (c + 1))
        in_maps.append({
            "xT": xT,
            "wq": _chunk_major(Wq[:, cols]).astype(bf16),
            "wk": _chunk_major(Wk[:, cols]).astype(bf16),
            "wv": _chunk_major(Wv[:, cols]).astype(bf16),
            "wo": np.ascontiguousarray(Wo[cols, :]).astype(bf16),
        })
    return in_maps


def combine_results(results):
    acc = np.zeros((C, L), np.float32)
    for r in results:
        acc += np.asarray(r["outT"], np.float32)
    return np.ascontiguousarray(acc.T)[None].astype(np.float32)


def kernel(x, Wq, Wk, Wv, Wo):
    from concourse.bass_utils import run_bass_kernel_spmd
    nc = _get_nc()
    in_maps = make_in_maps(x, Wq, Wk, Wv, Wo)
    res = run_bass_kernel_spmd(nc, in_maps, core_ids=list(range(N_CORES)))
    return combine_results(res.results)
